# revision 45
# baseline (speedup 1.0000x reference)
"""GCK 3x3 conv layer (nn_GCK3x3Layer) as a Trainium2 Bass kernel on 8 NeuronCores.

Math: out[o,h,w] = sum_{c,r,s} Weff[o,c,r,s] * x[c,h+r,w+s], where Weff is the
GCK linComb folded back through the +/-1 separable basis (done on host in f64).

Sharding: H split across 8 cores (64 output rows each, 66 input rows with halo).

Device scheme (col-tiled concurrent streams): input rows stored as even/odd
64-partition planes at the same free index.  Per block (row-pair pair: p0
normal, p1 psum-flipped), "main" K=128/M=64 matmuls for two output rows run
pairwise CONCURRENT in opposite PE column groups, and the four K=64
leftover-tap matmuls fill all four 64x64 array quadrants concurrently: 9 wall
slots per block.  Blocks are emitted in groups (GRP) with all mains before
all quads so the PE pays the mains<->quads row-group reconfiguration (~120ns)
once per group instead of once per block; group sizes are small at the start
(input-DMA paced) and end (output-DMA tail) and 4 in the middle (8 PSUM banks
= 4 blocks in flight; the warmup psum shares the rotation).

Compute dtype: float16 (10-bit mantissa), ~3.1e-4 rel err on the graded
seed-0 inputs; halves input HBM traffic vs f32.

Output: ALL f16 — per block one [128, 1024] f16 tile (ps0 via DVE CAST,
ps1 via ACT, in parallel, ~690ns each; DVE f32->f16 CAST measured SAME
speed as an f32 copy).  2KB DRAM lines; half the output bytes of the old
mixed f32/f16 scheme, and the final block's drain is 256KB instead of 512KB.

Input path: weights are packed INTO xin ([mains wts | slots 0-1 | quad wts
| slots 2-32]) so everything the first matmul needs rides ONE full-rate DMA
on the Sync ring.  A separate small weights DMA is ~2x slower per byte
(sub-2KB descriptors) and stalls the input stream; the ACT ring's first
packet also lags its doorbell by ~1.7us.  Per-SDMA-engine rate is ~25.5GB/s
with each engine serving 8 fixed partitions, so time-to-first-matmul is
bounded by (g0 bytes/partition)x8/25.5GB/s + ~0.7us issue + ~0.8us first
byte; g0 carries only what the first mains need.

Warmups: matmuls on a ZEROED tile (garbage operands pin the HAM clock to
0.7 GHz for the whole run!) must bridge program start to data-ready with NO
idle gap — any gap resets the HAM sustained-busy window (~3.4us) and the
1.2->2.4GHz unthrottle slips, costing ~1-2us of half-rate matmuls.  N=128
tail warmups keep the end-quantization fine.

Observed fixed overheads in the graded window (runtime, not kernel):
~1.3us entry + ~1.3us exit barrier + ~7us semaphore-reset teardown (resets
all 256 sems regardless of kernel).  HW exec ~49.0-49.5us cool device,
~50.0-50.5 after sustained benching (was 50.6us at session start); a deep
thermal-throttle state (PE pinned ~1.0GHz, exec ~59us) appears after many
back-to-back runs and clears with ~5min idle.  Timeline at 49.5us: head to
first real matmul 3.6us (issue 0.7 + first-byte 0.78 + g0 drain ~1.0 + sem/
warmup-quantization), PE stream 33.2us (floor 31.1 + ~1.1us mains<->quads
reconfigs + ~0.7us HAM-cold), tail 3.2us (copies 0.7 + issue 0.6 + first-
byte 0.78 + 256KB drain 0.7), fixed ~9.6us.
"""

import numpy as np

import concourse.bass as bass
import concourse.mybir as mybir
import concourse.tile as tile
from concourse import bacc
from concourse.bass_utils import run_bass_kernel_spmd

# Problem constants (hardcoded per contract)
C = 64          # input channels
O = 64          # output channels
H = W = 514     # input spatial
HO = WO = 512   # output spatial
NCORES = 8
ROWS_PER_CORE = HO // NCORES          # 64 output rows
PAIRS = ROWS_PER_CORE // 2            # 32 row pairs
JT = ROWS_PER_CORE // 2 + 1           # 33 input row-pair slots (incl. halo)
# Input DMA groups (row-pair slots per dma_start), ramped so the first matmul
# isn't gated on a large transfer competing with later queue-parallel loads.
XGS = [2, 3, 4, 5, 5, 5, 5, 4]        # sums to 33
XGO = [sum(XGS[:i]) for i in range(len(XGS))]
WM = 6 * 64                           # mains weights (mats 0-5), head of grp 0
WQ = 3 * 64                           # quad weights (mats 6-8), head of grp 1
# Weights ride at the head of the first two input groups so they arrive at
# full per-engine DMA rate with NO extra DMA issues ahead of group 0 (each
# issue costs ~0.65us of sequencer time before g0's descriptors hit the
# ring).  Mains weights + slots 0-1 gate the first matmul; quad weights
# aren't needed until ~6us later, so they ride group 1.
# xin layout: [wm | slot0 | slot1 | wq | slot2 | ... | slot32]
def _slot_off(j):
    """Column offset (elements) of row-pair slot j in the packed xin."""
    return WM + j * W + (WQ if j >= XGS[0] else 0)
# Warmup matmul free dims.  Dummy matmuls lift the PE HAM clock gate
# (1.2->2.4 GHz needs ~3.4us of sustained PE activity) while the first input
# DMA is in flight.  They must bridge from program start to data-ready
# (~program+3.2us) with NO idle gap (a gap resets the HAM busy window and
# delays the unthrottle), ending as close to data-ready as possible; the
# N=128 tail warmups (~107ns each cold) keep the overshoot quantization small.
WARMS = [320] * 9 + [128] * 5

V = np.array([[1.0, 1.0, 1.0], [1.0, -1.0, 1.0], [1.0, 1.0, -1.0]], dtype=np.float64)

MM_DT = mybir.dt.float16   # matmul operand dtype


def _fold_weights(linCombs: np.ndarray) -> np.ndarray:
    """linCombs (O, C*9) -> effective conv kernels Weff (O, C, 3, 3), f64."""
    L = linCombs.astype(np.float64).reshape(O, C, 3, 3)  # k = c*9 + 3i + j
    return np.einsum("ocij,ir,js->ocrs", L, V, V)


def _build_block_weights(Weff: np.ndarray) -> np.ndarray:
    """Weights for the col-tiled scheme, returned as [128, 9, 64] (k, idx, mu).

    idx 0..2  (s): K0=Wt(0,s), K1=Wt(1,s)  -- mains for EVEN output rows (rhs slot m)
    idx 3..5  (s): K0=Wt(1,s), K1=Wt(2,s)  -- mains for ODD  output rows (rhs slot m+1)
    idx 6..8  (s): K0=Wt(2,s) (leftover r2, even-plane rhs, partitions 0-63)
                   K1=Wt(0,s) (leftover r0, odd-plane rhs,  partitions 64-127)
    """
    Wt = {(r, s): Weff[:, :, r, s].T for r in range(3) for s in range(3)}  # [c, o]
    mats = np.zeros((9, 128, 64), dtype=np.float64)
    for s in range(3):
        mats[s, 0:64] = Wt[(0, s)]
        mats[s, 64:128] = Wt[(1, s)]
        mats[3 + s, 0:64] = Wt[(1, s)]
        mats[3 + s, 64:128] = Wt[(2, s)]
        mats[6 + s, 0:64] = Wt[(2, s)]
        mats[6 + s, 64:128] = Wt[(0, s)]
    return np.ascontiguousarray(mats.transpose(1, 0, 2).astype(np.float16))


def _build_program():
    nc = bacc.Bacc(None, target_bir_lowering=False, enable_partition_id=False)
    xin = nc.declare_dram_parameter(
        "xin", [128, WM + WQ + JT * W], MM_DT, isOutput=False
    )
    yout = nc.declare_dram_parameter(
        "yout", [PAIRS // 2, 128, 2 * WO], mybir.dt.float16, isOutput=True
    )

    with tile.TileContext(nc) as tc:
        with (
            tc.tile_pool(name="wpool", bufs=1) as wpool,
            tc.tile_pool(name="xpool", bufs=1) as xpool,
            tc.tile_pool(name="opool", bufs=4) as opool,
            tc.tile_pool(name="pspool", bufs=8, space="PSUM") as pspool,
        ):
            # PE warmup: matmuls on a zeroed scratch tile while the input
            # DMAs are in flight, so the HAM clock gate is partly ramped
            # (0.8->2.4 GHz) when the real matmuls start.  NOTE: the tile
            # MUST be zeroed — uninitialized SBUF garbage (NaNs/denormals)
            # keeps the HAM clock pinned at ~0.7 GHz for the entire run.
            warm = wpool.tile([128, max(WARMS)], mybir.dt.bfloat16, name="warm")
            nc.gpsimd.memset(warm[:], 0.0)
            # wps shares the ps rotation so its bank is recycled after warmup
            wps = pspool.tile([128, max(WARMS)], mybir.dt.float32, tag="ps", name="wps")
            for wn in WARMS:
                nc.tensor.matmul(
                    wps[:, :wn], warm[:, :128], warm[:, :wn], start=True, stop=True
                )

            total = WM + WQ + JT * W

            def gstart(g):  # column of group g's first element in xin
                if g == 0:
                    return 0
                return _slot_off(XGO[g]) - (WQ if g == 1 else 0)

            xgs = []
            for g, n in enumerate(XGS):
                g1 = gstart(g + 1) if g + 1 < len(XGS) else total
                xt = xpool.tile([128, g1 - gstart(g)], MM_DT,
                                tag=f"xt{g}", name=f"xt{g}")
                nc.sync.dma_start(xt[:], xin[:, gstart(g) : g1])
                xgs.append(xt)

            def rhs(j, s):
                for g in reversed(range(len(XGS))):
                    if j >= XGO[g]:
                        c = _slot_off(j) + s - gstart(g)
                        return xgs[g][:, c : c + WO]
                raise AssertionError(j)

            def wtv(i):  # weight mat i: mats 0-5 head grp 0, mats 6-8 head grp 1
                if i < 6:
                    return xgs[0][:, i * 64 : (i + 1) * 64]
                return xgs[1][:, (i - 6) * 64 : (i - 5) * 64]

            # Col-tiled concurrent-stream schedule: per block (p0=2t normal,
            # p1=2t+1 flipped), mains are 2-concurrent M=64 mms in opposite col
            # groups; the four K=64 leftovers fill all 4 array quadrants.
            # Blocks are emitted in PAIRS (mains t0, mains t1, quads t0, quads
            # t1) so the PE pays the mains<->quads row-group reconfiguration
            # cost once per two blocks instead of once per block.
            def emit_mains(ps0, ps1, p0, p1):
                for s in range(3):  # mains pair p0: out 2p0 @cg01, out 2p0+1 @cg23
                    nc.tensor.matmul(ps0[0:64, :], wtv(s), rhs(p0, s),
                                     start=(s == 0), stop=False)
                    nc.tensor.matmul(ps0[64:128, :], wtv(3 + s), rhs(p0 + 1, s),
                                     start=(s == 0), stop=False)
                for s in range(3):  # mains pair p1 (flipped): out 2p1 @cg23, out 2p1+1 @cg01
                    nc.tensor.matmul(ps1[64:128, :], wtv(s), rhs(p1, s),
                                     start=(s == 0), stop=False)
                    nc.tensor.matmul(ps1[0:64, :], wtv(3 + s), rhs(p1 + 1, s),
                                     start=(s == 0), stop=False)

            def emit_quads(ps0, ps1, p0, p1):
                for s in range(3):  # leftovers: 4 disjoint quadrants; ps1 first
                    e0 = rhs(p0 + 1, s)  # even half -> partitions 0-63
                    o0 = rhs(p0, s)      # odd half  -> partitions 64-127
                    e1 = rhs(p1 + 1, s)
                    o1 = rhs(p1, s)
                    nc.tensor.matmul(ps1[64:128, :], wtv(6 + s)[0:64, :], e1[0:64, :],
                                     start=False, stop=False)
                    nc.tensor.matmul(ps1[0:64, :], wtv(6 + s)[64:128, :], o1[64:128, :],
                                     start=False, stop=(s == 2))
                    nc.tensor.matmul(ps0[0:64, :], wtv(6 + s)[0:64, :], e0[0:64, :],
                                     start=False, stop=False)
                    nc.tensor.matmul(ps0[64:128, :], wtv(6 + s)[64:128, :], o0[64:128, :],
                                     start=False, stop=(s == 2))

            GRP = [2, 3, 4, 4, 2, 1]  # blocks per mains/quads phase group;
                                      # first groups small so cold mains
                                      # don't outrun the per-engine input
                                      # rate (~322ns/row-slot); last group =
                                      # 1 block so the final copies+DMAs
                                      # cover one block only ([2,4,4,4,2]
                                      # measured WORSE: a 3.7us sync-seq
                                      # stall delayed input groups)
            t0g = 0
            for gn in GRP:
                ts = list(range(t0g, t0g + gn))
                t0g += gn
                pss = {}
                for t in ts:
                    p0, p1 = 2 * t, 2 * t + 1
                    ps0 = pspool.tile([128, WO], mybir.dt.float32, tag="ps", name=f"ps{p0}")
                    ps1 = pspool.tile([128, WO], mybir.dt.float32, tag="ps", name=f"ps{p1}")
                    pss[t] = (ps0, ps1)
                    emit_mains(ps0, ps1, p0, p1)
                for t in ts:
                    emit_quads(*pss[t], 2 * t, 2 * t + 1)
                for t in ts:
                    # All-f16 output: DVE CAST f32->f16 runs at the same
                    # ~690ns as an f32 copy (measured; the old "DVE f16 cast
                    # is 2.5us" lore is wrong on current firmware), so ps0
                    # (DVE) and ps1 (ACT) convert in parallel into one
                    # [128, 1024] f16 tile = 2KB DRAM lines, half the output
                    # bytes of the old mixed f32/f16 scheme.
                    ps0, ps1 = pss[t]
                    ot = opool.tile([128, 2 * WO], mybir.dt.float16, tag="ot",
                                    name=f"ot{t}")
                    nc.scalar.copy(ot[:, WO:], ps1[:])
                    nc.vector.tensor_copy(ot[:, :WO], ps0[:])
                    # issues on the Sync sequencer (idle after the input
                    # groups); the Scalar sequencer is busy with the ACT
                    # copies.  2KB lines, one whole-tile DMA per block.
                    # Final block: scalar ring (empty), so its issue starts
                    # the instant its copies land instead of queueing behind
                    # t14's sync issue.
                    if t == PAIRS // 2 - 1:
                        nc.scalar.dma_start(yout[t], ot[:])
                    else:
                        nc.sync.dma_start(yout[t], ot[:])

    nc.compile()
    return nc


_NC_CACHE = None


def _make_in_maps(x0: np.ndarray, wts_h: np.ndarray) -> list:
    """Per-core xin: [mains wts | slots 0-1 | quad wts | slots 2-32] (f16)."""
    wm = np.ascontiguousarray(wts_h[:, 0:6].reshape(128, WM).astype(np.float16))
    wq = np.ascontiguousarray(wts_h[:, 6:9].reshape(128, WQ).astype(np.float16))
    n0 = XGS[0]
    in_maps = []
    for core in range(NCORES):
        r0 = core * ROWS_PER_CORE
        ev = x0[:, r0:r0 + 2 * JT:2, :]       # [64, 33, W] even local rows
        od = x0[:, r0 + 1:r0 + 2 * JT:2, :]   # [64, 33, W] odd local rows
        planes = np.concatenate([ev, od], axis=0).astype(np.float16)
        P = np.ascontiguousarray(np.concatenate([
            wm,
            planes[:, :n0].reshape(128, n0 * W),
            wq,
            planes[:, n0:].reshape(128, (JT - n0) * W),
        ], axis=1))
        in_maps.append({"xin": P})
    return in_maps


def kernel(input: np.ndarray, linCombs: np.ndarray) -> np.ndarray:
    global _NC_CACHE
    x = np.ascontiguousarray(np.asarray(input, dtype=np.float32))
    L = np.asarray(linCombs, dtype=np.float32)
    assert x.shape == (1, C, H, W), x.shape

    Weff = _fold_weights(L)
    wts_h = _build_block_weights(Weff)

    in_maps = _make_in_maps(x[0], wts_h)

    if _NC_CACHE is None:
        _NC_CACHE = _build_program()
    res = run_bass_kernel_spmd(_NC_CACHE, in_maps, list(range(NCORES)))

    out = np.empty((1, O, HO, WO), dtype=np.float32)
    for core in range(NCORES):
        # yout[t] = [128, 1024] f16: [:, :512] = ps0 (rows 4t+par),
        # [:, 512:] = ps1 (flipped: rows 4t+2+(1-par))
        y = res.results[core]["yout"].astype(np.float32)
        y = y.reshape(PAIRS // 2, 2, O, 2, WO)                  # [t, par, o, half, w]
        r0 = core * ROWS_PER_CORE
        for par in range(2):
            out[0, :, r0 + par : r0 + ROWS_PER_CORE : 4, :] = \
                y[:, par, :, 0, :].transpose(1, 0, 2)
            out[0, :, r0 + 2 + (1 - par) : r0 + ROWS_PER_CORE : 4, :] = \
                y[:, par, :, 1, :].transpose(1, 0, 2)
    return out



# revision 46
# speedup vs baseline: 1.0046x; 1.0046x over previous
"""GCK 3x3 conv layer (nn_GCK3x3Layer) as a Trainium2 Bass kernel on 8 NeuronCores.

Math: out[o,h,w] = sum_{c,r,s} Weff[o,c,r,s] * x[c,h+r,w+s], where Weff is the
GCK linComb folded back through the +/-1 separable basis (done on host in f64).

Sharding: H split across 8 cores (64 output rows each, 66 input rows with halo).

Device scheme (col-tiled concurrent streams): input rows stored as even/odd
64-partition planes at the same free index.  Per block (row-pair pair: p0
normal, p1 psum-flipped), "main" K=128/M=64 matmuls for two output rows run
pairwise CONCURRENT in opposite PE column groups, and the four K=64
leftover-tap matmuls fill all four 64x64 array quadrants concurrently: 9 wall
slots per block.  Blocks are emitted in groups (GRP) with all mains before
all quads so the PE pays the mains<->quads row-group reconfiguration (~120ns)
once per group instead of once per block; group sizes are small at the start
(input-DMA paced) and end (output-DMA tail) and 4 in the middle (8 PSUM banks
= 4 blocks in flight; the warmup psum shares the rotation).

Compute dtype: float16 (10-bit mantissa), ~3.1e-4 rel err on the graded
seed-0 inputs; halves input HBM traffic vs f32.

Output: ALL f16 — per block one [128, 1024] f16 tile (ps0 via DVE CAST,
ps1 via ACT, in parallel, ~690ns each; DVE f32->f16 CAST measured SAME
speed as an f32 copy).  2KB DRAM lines; half the output bytes of the old
mixed f32/f16 scheme, and the final block's drain is 256KB instead of 512KB.

Input path: weights are packed INTO xin ([mains wts | slots 0-1 | quad wts
| slots 2-32]) so everything the first matmul needs rides ONE full-rate DMA
on the Sync ring.  A separate small weights DMA is ~2x slower per byte
(sub-2KB descriptors) and stalls the input stream; the ACT ring's first
packet also lags its doorbell by ~1.7us.  Per-SDMA-engine rate is ~25.5GB/s
with each engine serving 8 fixed partitions, so time-to-first-matmul is
bounded by (g0 bytes/partition)x8/25.5GB/s + ~0.7us issue + ~0.8us first
byte; g0 carries only what the first mains need.

Warmups: matmuls on a ZEROED tile (garbage operands pin the HAM clock to
0.7 GHz for the whole run!) must bridge program start to data-ready with NO
idle gap — any gap resets the HAM sustained-busy window (~3.4us) and the
1.2->2.4GHz unthrottle slips, costing ~1-2us of half-rate matmuls.  N=128
tail warmups keep the end-quantization fine.

Observed fixed overheads in the graded window (runtime, not kernel):
~1.3us entry + ~1.3us exit barrier + ~7us semaphore-reset teardown (resets
all 256 sems regardless of kernel).  HW exec ~49.0-49.5us cool device,
~50.0-50.5 after sustained benching (was 50.6us at session start); a deep
thermal-throttle state (PE pinned ~1.0GHz, exec ~59us) appears after many
back-to-back runs and clears with ~5min idle.  Timeline at 49.5us: head to
first real matmul 3.6us (issue 0.7 + first-byte 0.78 + g0 drain ~1.0 + sem/
warmup-quantization), PE stream 33.2us (floor 31.1 + ~1.1us mains<->quads
reconfigs + ~0.7us HAM-cold), tail 3.2us (copies 0.7 + issue 0.6 + first-
byte 0.78 + 256KB drain 0.7), fixed ~9.6us.
"""

import numpy as np

import concourse.bass as bass
import concourse.mybir as mybir
import concourse.tile as tile
from concourse import bacc
from concourse.bass_utils import run_bass_kernel_spmd

# Problem constants (hardcoded per contract)
C = 64          # input channels
O = 64          # output channels
H = W = 514     # input spatial
HO = WO = 512   # output spatial
NCORES = 8
ROWS_PER_CORE = HO // NCORES          # 64 output rows
PAIRS = ROWS_PER_CORE // 2            # 32 row pairs
JT = ROWS_PER_CORE // 2 + 1           # 33 input row-pair slots (incl. halo)
# Input DMA groups (row-pair slots per dma_start), ramped so the first matmul
# isn't gated on a large transfer competing with later queue-parallel loads.
XGS = [2, 3, 4, 5, 5, 5, 5, 4]        # sums to 33
XGO = [sum(XGS[:i]) for i in range(len(XGS))]
WM = 6 * 64                           # mains weights (mats 0-5), head of grp 0
WQ = 3 * 64                           # quad weights (mats 6-8), head of grp 1
# Weights ride at the head of the first two input groups so they arrive at
# full per-engine DMA rate with NO extra DMA issues ahead of group 0 (each
# issue costs ~0.65us of sequencer time before g0's descriptors hit the
# ring).  Mains weights + slots 0-1 gate the first matmul; quad weights
# aren't needed until ~6us later, so they ride group 1.
# xin layout: [wm | slot0 | slot1 | wq | slot2 | ... | slot32]
def _slot_off(j):
    """Column offset (elements) of row-pair slot j in the packed xin."""
    return WM + j * W + (WQ if j >= XGS[0] else 0)
# Warmup matmul free dims.  Dummy matmuls lift the PE HAM clock gate
# (1.2->2.4 GHz needs ~3.4us of sustained PE activity) while the first input
# DMA is in flight.  They must bridge from program start to data-ready
# (~program+3.2us) with NO idle gap (a gap resets the HAM busy window and
# delays the unthrottle), ending as close to data-ready as possible; the
# N=128 tail warmups (~107ns each cold) keep the overshoot quantization small.
WARMS = [320] * 9 + [128] * 5

V = np.array([[1.0, 1.0, 1.0], [1.0, -1.0, 1.0], [1.0, 1.0, -1.0]], dtype=np.float64)

MM_DT = mybir.dt.float16   # matmul operand dtype


def _fold_weights(linCombs: np.ndarray) -> np.ndarray:
    """linCombs (O, C*9) -> effective conv kernels Weff (O, C, 3, 3), f64."""
    L = linCombs.astype(np.float64).reshape(O, C, 3, 3)  # k = c*9 + 3i + j
    return np.einsum("ocij,ir,js->ocrs", L, V, V)


def _build_block_weights(Weff: np.ndarray) -> np.ndarray:
    """Weights for the col-tiled scheme, returned as [128, 9, 64] (k, idx, mu).

    idx 0..2  (s): K0=Wt(0,s), K1=Wt(1,s)  -- mains for EVEN output rows (rhs slot m)
    idx 3..5  (s): K0=Wt(1,s), K1=Wt(2,s)  -- mains for ODD  output rows (rhs slot m+1)
    idx 6..8  (s): K0=Wt(2,s) (leftover r2, even-plane rhs, partitions 0-63)
                   K1=Wt(0,s) (leftover r0, odd-plane rhs,  partitions 64-127)
    """
    Wt = {(r, s): Weff[:, :, r, s].T for r in range(3) for s in range(3)}  # [c, o]
    mats = np.zeros((9, 128, 64), dtype=np.float64)
    for s in range(3):
        mats[s, 0:64] = Wt[(0, s)]
        mats[s, 64:128] = Wt[(1, s)]
        mats[3 + s, 0:64] = Wt[(1, s)]
        mats[3 + s, 64:128] = Wt[(2, s)]
        mats[6 + s, 0:64] = Wt[(2, s)]
        mats[6 + s, 64:128] = Wt[(0, s)]
    return np.ascontiguousarray(mats.transpose(1, 0, 2).astype(np.float16))


def _build_program():
    nc = bacc.Bacc(None, target_bir_lowering=False, enable_partition_id=False)
    xin = nc.declare_dram_parameter(
        "xin", [128, WM + WQ + JT * W], MM_DT, isOutput=False
    )
    yout = nc.declare_dram_parameter(
        "yout", [PAIRS // 2, 128, 2 * WO], mybir.dt.float16, isOutput=True
    )

    with tile.TileContext(nc) as tc:
        with (
            tc.tile_pool(name="wpool", bufs=1) as wpool,
            tc.tile_pool(name="xpool", bufs=1) as xpool,
            tc.tile_pool(name="opool", bufs=4) as opool,
            tc.tile_pool(name="pspool", bufs=8, space="PSUM") as pspool,
        ):
            # PE warmup: matmuls on a zeroed scratch tile while the input
            # DMAs are in flight, so the HAM clock gate is partly ramped
            # (0.8->2.4 GHz) when the real matmuls start.  NOTE: the tile
            # MUST be zeroed — uninitialized SBUF garbage (NaNs/denormals)
            # keeps the HAM clock pinned at ~0.7 GHz for the entire run.
            warm = wpool.tile([128, max(WARMS)], mybir.dt.bfloat16, name="warm")
            nc.gpsimd.memset(warm[:], 0.0)
            # wps shares the ps rotation so its bank is recycled after warmup
            wps = pspool.tile([128, max(WARMS)], mybir.dt.float32, tag="ps", name="wps")
            for wn in WARMS:
                nc.tensor.matmul(
                    wps[:, :wn], warm[:, :128], warm[:, :wn], start=True, stop=True
                )

            total = WM + WQ + JT * W

            def gstart(g):  # column of group g's first element in xin
                if g == 0:
                    return 0
                return _slot_off(XGO[g]) - (WQ if g == 1 else 0)

            xgs = []
            for g, n in enumerate(XGS):
                g1 = gstart(g + 1) if g + 1 < len(XGS) else total
                xt = xpool.tile([128, g1 - gstart(g)], MM_DT,
                                tag=f"xt{g}", name=f"xt{g}")
                nc.sync.dma_start(xt[:], xin[:, gstart(g) : g1])
                xgs.append(xt)

            def rhs(j, s):
                for g in reversed(range(len(XGS))):
                    if j >= XGO[g]:
                        c = _slot_off(j) + s - gstart(g)
                        return xgs[g][:, c : c + WO]
                raise AssertionError(j)

            def wtv(i):  # weight mat i: mats 0-5 head grp 0, mats 6-8 head grp 1
                if i < 6:
                    return xgs[0][:, i * 64 : (i + 1) * 64]
                return xgs[1][:, (i - 6) * 64 : (i - 5) * 64]

            # Col-tiled concurrent-stream schedule: per block (p0=2t normal,
            # p1=2t+1 flipped), mains are 2-concurrent M=64 mms in opposite col
            # groups; the four K=64 leftovers fill all 4 array quadrants.
            # Blocks are emitted in PAIRS (mains t0, mains t1, quads t0, quads
            # t1) so the PE pays the mains<->quads row-group reconfiguration
            # cost once per two blocks instead of once per block.
            def emit_mains(ps0, ps1, p0, p1):
                for s in range(3):  # mains pair p0: out 2p0 @cg01, out 2p0+1 @cg23
                    nc.tensor.matmul(ps0[0:64, :], wtv(s), rhs(p0, s),
                                     start=(s == 0), stop=False)
                    nc.tensor.matmul(ps0[64:128, :], wtv(3 + s), rhs(p0 + 1, s),
                                     start=(s == 0), stop=False)
                for s in range(3):  # mains pair p1 (flipped): out 2p1 @cg23, out 2p1+1 @cg01
                    nc.tensor.matmul(ps1[64:128, :], wtv(s), rhs(p1, s),
                                     start=(s == 0), stop=False)
                    nc.tensor.matmul(ps1[0:64, :], wtv(3 + s), rhs(p1 + 1, s),
                                     start=(s == 0), stop=False)

            def emit_quads(ps0, ps1, p0, p1):
                for s in range(3):  # leftovers: 4 disjoint quadrants; ps1 first
                    e0 = rhs(p0 + 1, s)  # even half -> partitions 0-63
                    o0 = rhs(p0, s)      # odd half  -> partitions 64-127
                    e1 = rhs(p1 + 1, s)
                    o1 = rhs(p1, s)
                    nc.tensor.matmul(ps1[64:128, :], wtv(6 + s)[0:64, :], e1[0:64, :],
                                     start=False, stop=False)
                    nc.tensor.matmul(ps1[0:64, :], wtv(6 + s)[64:128, :], o1[64:128, :],
                                     start=False, stop=(s == 2))
                    nc.tensor.matmul(ps0[0:64, :], wtv(6 + s)[0:64, :], e0[0:64, :],
                                     start=False, stop=False)
                    nc.tensor.matmul(ps0[64:128, :], wtv(6 + s)[64:128, :], o0[64:128, :],
                                     start=False, stop=(s == 2))

            GRP = [2, 3, 4, 4, 2, 1]  # blocks per mains/quads phase group;
                                      # first groups small so cold mains
                                      # don't outrun the per-engine input
                                      # rate (~322ns/row-slot); last group =
                                      # 1 block so the final copies+DMAs
                                      # cover one block only ([2,4,4,4,2]
                                      # measured WORSE: a 3.7us sync-seq
                                      # stall delayed input groups)
            t0g = 0
            for gn in GRP:
                ts = list(range(t0g, t0g + gn))
                t0g += gn
                pss = {}
                for t in ts:
                    p0, p1 = 2 * t, 2 * t + 1
                    ps0 = pspool.tile([128, WO], mybir.dt.float32, tag="ps", name=f"ps{p0}")
                    ps1 = pspool.tile([128, WO], mybir.dt.float32, tag="ps", name=f"ps{p1}")
                    pss[t] = (ps0, ps1)
                    emit_mains(ps0, ps1, p0, p1)
                for t in ts:
                    emit_quads(*pss[t], 2 * t, 2 * t + 1)
                for t in ts:
                    # All-f16 output: DVE CAST f32->f16 runs at the same
                    # ~690ns as an f32 copy (measured; the old "DVE f16 cast
                    # is 2.5us" lore is wrong on current firmware), so ps0
                    # (DVE) and ps1 (ACT) convert in parallel into one
                    # [128, 1024] f16 tile = 2KB DRAM lines, half the output
                    # bytes of the old mixed f32/f16 scheme.
                    ps0, ps1 = pss[t]
                    ot = opool.tile([128, 2 * WO], mybir.dt.float16, tag="ot",
                                    name=f"ot{t}")
                    nc.scalar.copy(ot[:, WO:], ps1[:])
                    nc.vector.tensor_copy(ot[:, :WO], ps0[:])
                    # issues on the Sync sequencer (idle after the input
                    # groups); the Scalar sequencer is busy with the ACT
                    # copies.  2KB lines, one whole-tile DMA per block.
                    # Final block: split on the partition axis across BOTH
                    # HWDGE rings — partitions 0-63/64-127 map to disjoint
                    # even/odd SDMA engines (port swizzle), so two
                    # 64-descriptor issues (~0.32us each, on two sequencers)
                    # replace one 0.62us issue and the halves drain in
                    # parallel.  Lines stay 2KB (full packet rate).
                    if t == PAIRS // 2 - 1:
                        nc.scalar.dma_start(yout[t][0:64, :], ot[0:64, :])
                        nc.sync.dma_start(yout[t][64:128, :], ot[64:128, :])
                    else:
                        nc.sync.dma_start(yout[t], ot[:])

    nc.compile()
    return nc


_NC_CACHE = None


def _make_in_maps(x0: np.ndarray, wts_h: np.ndarray) -> list:
    """Per-core xin: [mains wts | slots 0-1 | quad wts | slots 2-32] (f16)."""
    wm = np.ascontiguousarray(wts_h[:, 0:6].reshape(128, WM).astype(np.float16))
    wq = np.ascontiguousarray(wts_h[:, 6:9].reshape(128, WQ).astype(np.float16))
    n0 = XGS[0]
    in_maps = []
    for core in range(NCORES):
        r0 = core * ROWS_PER_CORE
        ev = x0[:, r0:r0 + 2 * JT:2, :]       # [64, 33, W] even local rows
        od = x0[:, r0 + 1:r0 + 2 * JT:2, :]   # [64, 33, W] odd local rows
        planes = np.concatenate([ev, od], axis=0).astype(np.float16)
        P = np.ascontiguousarray(np.concatenate([
            wm,
            planes[:, :n0].reshape(128, n0 * W),
            wq,
            planes[:, n0:].reshape(128, (JT - n0) * W),
        ], axis=1))
        in_maps.append({"xin": P})
    return in_maps


def kernel(input: np.ndarray, linCombs: np.ndarray) -> np.ndarray:
    global _NC_CACHE
    x = np.ascontiguousarray(np.asarray(input, dtype=np.float32))
    L = np.asarray(linCombs, dtype=np.float32)
    assert x.shape == (1, C, H, W), x.shape

    Weff = _fold_weights(L)
    wts_h = _build_block_weights(Weff)

    in_maps = _make_in_maps(x[0], wts_h)

    if _NC_CACHE is None:
        _NC_CACHE = _build_program()
    res = run_bass_kernel_spmd(_NC_CACHE, in_maps, list(range(NCORES)))

    out = np.empty((1, O, HO, WO), dtype=np.float32)
    for core in range(NCORES):
        # yout[t] = [128, 1024] f16: [:, :512] = ps0 (rows 4t+par),
        # [:, 512:] = ps1 (flipped: rows 4t+2+(1-par))
        y = res.results[core]["yout"].astype(np.float32)
        y = y.reshape(PAIRS // 2, 2, O, 2, WO)                  # [t, par, o, half, w]
        r0 = core * ROWS_PER_CORE
        for par in range(2):
            out[0, :, r0 + par : r0 + ROWS_PER_CORE : 4, :] = \
                y[:, par, :, 0, :].transpose(1, 0, 2)
            out[0, :, r0 + 2 + (1 - par) : r0 + ROWS_PER_CORE : 4, :] = \
                y[:, par, :, 1, :].transpose(1, 0, 2)
    return out

